# revision 6
# baseline (speedup 1.0000x reference)
"""Causal self-attention (B=2048, T=128, C=192, H=6, D=32) on 8 TRN2 cores.

Data-parallel over batch: 256 elems/core. v3: quad-batched qkv (N=512
matmuls), single fused exp per pair, broadcast-AP softmax normalize (one
tensor_tensor instead of 12 tensor_scalars), reduce/copies balanced across
scalar/vector/gpsimd engines.

Per quad (4 elems):
  x --DMA--> xf --cast--> x16 --PE transpose--> xT (+ones row)
  qT/kT = W^T @ xT (N=512, bias fused);  v = xT^T @ Wv (bias fused)
Per pair (2 elems):
  S_h[t,s] = q_h^T k_h (row-tiled PE, 4-concurrent)
  P = exp(S) one ACTIVATE;  Pm = P*tril (bcast mul);  rsum (DVE reduce);
  rrec duplicated-pair recip;  Pn = Pm*rrec_bcast (one mul, 2x mode);
  P^T via PE transpose;  y^T = V^T P^T (col-tiled);  out = y W_p -> HBM.
"""

import sys

sys.path.insert(0, "/opt/trn_rl_repo")

import numpy as np
import ml_dtypes

N_CORES = 8
B, T, C = 2048, 128, 192
NH, HD = 6, 32
BL = B // N_CORES  # 256 per core

_CACHE = {}


def _build(bl):
    from contextlib import ExitStack

    import concourse.bass as bass
    import concourse.mybir as mybir
    import concourse.tile as tile
    from concourse import bacc

    fp32 = mybir.dt.float32
    bf16 = mybir.dt.bfloat16
    AF = mybir.ActivationFunctionType

    nc = bacc.Bacc("TRN2", target_bir_lowering=False, debug=False)

    x_d = nc.dram_tensor("x", [bl, T, C], fp32, kind="ExternalInput")
    wA_d = nc.dram_tensor("wA", [128, 704], bf16, kind="ExternalInput")
    wB_d = nc.dram_tensor("wB", [65, 704], bf16, kind="ExternalInput")
    wpA_d = nc.dram_tensor("wpA", [128, 192], bf16, kind="ExternalInput")
    wpB_d = nc.dram_tensor("wpB", [65, 192], bf16, kind="ExternalInput")
    tril_d = nc.dram_tensor("trilR", [128, 6, 128], bf16, kind="ExternalInput")
    idr_d = nc.dram_tensor("identR", [128, 128], bf16, kind="ExternalInput")
    out_d = nc.dram_tensor("out", [bl, T, C], fp32, kind="ExternalOutput")

    with tile.TileContext(nc) as tc, ExitStack() as ctx:
        consts = ctx.enter_context(tc.tile_pool(name="consts", bufs=1))
        sbq = ctx.enter_context(tc.tile_pool(name="sbq", bufs=2))
        sbp = ctx.enter_context(tc.tile_pool(name="sbp", bufs=3))
        ps = ctx.enter_context(
            tc.tile_pool(name="ps", bufs=1, space=bass.MemorySpace.PSUM)
        )

        wA = consts.tile([128, 704], bf16)
        nc.sync.dma_start(wA[:], wA_d[:])
        wB = consts.tile([65, 704], bf16)
        nc.sync.dma_start(wB[:], wB_d[:])
        wpA = consts.tile([128, 192], bf16)
        nc.sync.dma_start(wpA[:], wpA_d[:])
        wpB = consts.tile([65, 192], bf16)
        nc.sync.dma_start(wpB[:], wpB_d[:])
        trilR = consts.tile([128, 6, 128], bf16)
        nc.sync.dma_start(trilR[:], tril_d[:])
        ident = consts.tile([128, 128], bf16)
        nc.sync.dma_start(ident[:], idr_d[:])

        def pt(tag, shape, dtype=fp32, name=None):
            return ps.tile(shape, dtype, tag=tag, name=name or f"ps_{tag}")

        for q in range(bl // 4):
            # ---------------- quad phase: load, transpose, qkv ----------
            xf = sbq.tile([128, 4, 192], fp32, tag="xf")
            nc.sync.dma_start(
                xf[:], x_d[4 * q : 4 * q + 4].rearrange("e t c -> t e c")
            )
            x16 = sbq.tile([128, 4, 256], bf16, tag="x16")
            nc.vector.tensor_copy(x16[:, :, 0:192], xf[:])

            xTp = pt("xt1", [128, 4, 2, 128], bf16)
            for e in range(4):
                nc.tensor.transpose(xTp[:, e, 0, :], x16[:, e, 0:128], ident[:])
                nc.tensor.transpose(xTp[:, e, 1, :], x16[:, e, 128:256], ident[:])
            xT = sbq.tile([128, 4, 2, 128], bf16, tag="xT")
            nc.vector.tensor_copy(xT[:], xTp[:])
            nc.gpsimd.memset(xT[64:65, :, 1, :], 1.0)

            # T6: 6-bank psum tile. Banks 0-3: qkT j-blocks
            # [q h0-3 | q h4-5 | k h0-3 | k h4-5]; banks 4-5: v (2 elems each)
            T6 = pt("big6", [128, 6, 4, 128])
            for j in range(4):
                nc.tensor.matmul(
                    T6[:, j, :, :],
                    wA[:, 128 * j : 128 * (j + 1)],
                    xT[:, :, 0, :],
                    start=True,
                    stop=False,
                )
                nc.tensor.matmul(
                    T6[:, j, :, :],
                    wB[:, 128 * j : 128 * (j + 1)],
                    xT[0:65, :, 1, :],
                    start=False,
                    stop=True,
                )
            for e in range(4):
                vslot = T6[:, 4 + e // 2, 2 * (e % 2) : 2 * (e % 2) + 2, :]
                vout = vslot.rearrange("p a b -> p (a b)")[:, 0:192]
                nc.tensor.matmul(
                    vout,
                    xT[:, e, 0, :],
                    wA[:, 512:704],
                    start=True,
                    stop=False,
                )
                nc.tensor.matmul(
                    vout,
                    xT[0:65, e, 1, :],
                    wB[:, 512:704],
                    start=False,
                    stop=True,
                )
            qkT = sbq.tile([128, 4, 4, 128], bf16, tag="qkT")
            nc.scalar.copy(qkT[:], T6[:, 0:4, :, :])
            v16 = sbq.tile([128, 4, 192], bf16, tag="v16")
            nc.scalar.copy(
                v16[:],
                T6[:, 4:6, :, :]
                .rearrange("p a b c -> p (a b c)")
                .rearrange("p (e c) -> p e c", c=256)[:, :, 0:192],
            )

            # ---------------- pair phase: attention core ----------------
            for half in range(2):
                e0 = 2 * half  # elems e0, e0+1 of this quad

                # S[h, ee]: bank h*2+ee)//2 -> concurrent heads in distinct banks
                S = pt("big6", [128, 6, 2, 256], name=f"S_{half}")
                for ee in range(2):
                    e = e0 + ee
                    for h in range(NH):
                        r = (h % 4) * 32
                        jq, jk = (0, 2) if h < 4 else (1, 3)
                        nc.tensor.matmul(
                            S[:, h, ee, 0:128],
                            qkT[r : r + 32, jq, e, :],
                            qkT[r : r + 32, jk, e, :],
                            start=True,
                            stop=True,
                            tile_position=(r, 0),
                        )

                P16 = sbp.tile([128, 6, 2, 128], bf16, tag="P16")
                nc.scalar.activation(P16[:], S[:, :, :, 0:128], AF.Exp)

                Pm = sbp.tile([128, 6, 2, 128], bf16, tag="Pm")
                for ee in range(2):
                    nc.vector.tensor_mul(
                        Pm[:, :, ee, :], P16[:, :, ee, :], trilR[:]
                    )
                rsum = sbp.tile([128, 6, 2], fp32, tag="rsum")
                nc.vector.reduce_sum(rsum[:], Pm[:], axis=mybir.AxisListType.X)
                rrec = sbp.tile([128, 6, 2], fp32, tag="rrec")
                nc.vector.reciprocal(rrec[:], rsum[:])
                Pn = sbp.tile([128, 6, 2, 128], bf16, tag="Pn")
                for ee in range(2):
                    for h in range(NH):
                        nc.vector.tensor_scalar_mul(
                            Pn[:, h, ee, :],
                            Pm[:, h, ee, :],
                            rrec[:, h, ee : ee + 1],
                        )

                PTp = pt("xt1", [128, 6, 2, 128], bf16, name=f"PTp_{half}")
                for ee in range(2):
                    for h in range(NH):
                        nc.tensor.transpose(
                            PTp[:, h, ee, :], Pn[:, h, ee, :], ident[:]
                        )
                PT = sbp.tile([128, 6, 2, 128], bf16, tag="PT")
                nc.scalar.copy(PT[:], PTp[:])

                yt = pt("xt1", [128, 2, 2, 128], name=f"yt_{half}")
                for ee in range(2):
                    e = e0 + ee
                    for h in range(NH):
                        r = (h % 4) * 32
                        j = 0 if h < 4 else 1
                        nc.tensor.matmul(
                            yt[r : r + 32, ee, j, :],
                            v16[:, e, h * 32 : h * 32 + 32],
                            PT[:, h, ee, :],
                            start=True,
                            stop=True,
                            tile_position=(0, r),
                        )
                yT = sbp.tile([128, 2, 2, 128], bf16, tag="yT")
                nc.vector.tensor_copy(yT[:, :, 0, :], yt[:, :, 0, :])
                nc.vector.tensor_copy(yT[0:64, :, 1, :], yt[0:64, :, 1, :])
                nc.gpsimd.memset(yT[64:65, :, 1, :], 1.0)

                outs = sbp.tile([128, 2, 192], fp32, tag="outs")
                for ee in range(2):
                    outp = pt("xt1", [128, 192], name=f"outp_{half}_{ee}")
                    nc.tensor.matmul(
                        outp[:], yT[:, ee, 0, :], wpA[:], start=True, stop=False
                    )
                    nc.tensor.matmul(
                        outp[:],
                        yT[0:65, ee, 1, :],
                        wpB[:],
                        start=False,
                        stop=True,
                    )
                    nc.scalar.copy(outs[:, ee, :], outp[:])
                nc.sync.dma_start(
                    out_d[4 * q + e0 : 4 * q + e0 + 2].rearrange(
                        "e t c -> t e c"
                    ),
                    outs[:],
                )

    nc.finalize()
    return nc


def _prep_inputs(x, w_qkv, b_qkv, w_proj, b_proj, bl):
    bf = ml_dtypes.bfloat16
    scale = 1.0 / np.sqrt(HD)
    w2 = np.array(w_qkv, dtype=np.float32, copy=True)
    b2 = np.array(b_qkv, dtype=np.float32, copy=True)
    w2[:, 0:C] *= scale
    b2[0:C] *= scale
    # column order: [q h0-3 | q h4-5 + pad | k h0-3 | k h4-5 + pad | v]
    # (pad cols produce junk in unread partitions, keeping M=128 full-mode)
    perm = np.concatenate(
        [
            np.arange(0, 128),
            np.arange(128, 192),
            np.arange(0, 64),
            np.arange(192, 320),
            np.arange(320, 384),
            np.arange(0, 64),
            np.arange(384, 576),
        ]
    )
    wA = w2[0:128][:, perm].astype(bf)
    wB = np.concatenate([w2[128:192], b2[None, :]], axis=0)[:, perm].astype(bf)
    wpA = np.asarray(w_proj)[0:128].astype(bf)
    wpB = np.concatenate(
        [np.asarray(w_proj)[128:192], np.asarray(b_proj)[None, :]], axis=0
    ).astype(bf)
    trilR = np.ascontiguousarray(
        np.broadcast_to(
            np.tril(np.ones((128, 128), np.float32)), (6, 128, 128)
        ).transpose(1, 0, 2)
    ).astype(bf)
    identR = np.eye(128, dtype=np.float32).astype(bf)
    xs = np.ascontiguousarray(np.asarray(x, dtype=np.float32)).reshape(
        -1, bl, T, C
    )
    maps = []
    for i in range(xs.shape[0]):
        maps.append(
            {
                "x": xs[i],
                "wA": wA,
                "wB": wB,
                "wpA": wpA,
                "wpB": wpB,
                "trilR": trilR,
                "identR": identR,
            }
        )
    return maps


def _run(x, w_qkv, b_qkv, w_proj, b_proj, bl=BL, n_cores=N_CORES, trace=False):
    from concourse.bass_utils import run_bass_kernel_spmd

    key = bl
    if key not in _CACHE:
        _CACHE[key] = _build(bl)
    nc = _CACHE[key]
    maps = _prep_inputs(x, w_qkv, b_qkv, w_proj, b_proj, bl)[:n_cores]
    res = run_bass_kernel_spmd(
        nc, maps, core_ids=list(range(len(maps))), trace=trace
    )
    out = np.concatenate([r["out"] for r in res.results], axis=0)
    return out, res


def kernel(x, w_qkv, b_qkv, w_proj, b_proj):
    out, _ = _run(x, w_qkv, b_qkv, w_proj, b_proj)
    return out.reshape(B, T, C).astype(np.float32)


# revision 14
# speedup vs baseline: 1.9884x; 1.9884x over previous
"""Causal self-attention (B=2048, T=128, C=192, H=6, D=32) on 8 TRN2 cores.

Data-parallel over batch: 256 elems/core. v3: quad-batched qkv (N=512
matmuls), single fused exp per pair, broadcast-AP softmax normalize (one
tensor_tensor instead of 12 tensor_scalars), reduce/copies balanced across
scalar/vector/gpsimd engines.

Per quad (4 elems):
  x --DMA--> xf --cast--> x16 --PE transpose--> xT (+ones row)
  qT/kT = W^T @ xT (N=512, bias fused);  v = xT^T @ Wv (bias fused)
Per pair (2 elems):
  S_h[t,s] = q_h^T k_h (row-tiled PE, 4-concurrent)
  P = exp(S) one ACTIVATE;  Pm = P*tril (bcast mul);  rsum (DVE reduce);
  rrec duplicated-pair recip;  Pn = Pm*rrec_bcast (one mul, 2x mode);
  P^T via PE transpose;  y^T = V^T P^T (col-tiled);  out = y W_p -> HBM.
"""

import sys

sys.path.insert(0, "/opt/trn_rl_repo")

import numpy as np
import ml_dtypes

N_CORES = 8
B, T, C = 2048, 128, 192
NH, HD = 6, 32
BL = B // N_CORES  # 256 per core

_CACHE = {}


def _build(bl):
    from contextlib import ExitStack

    import concourse.bass as bass
    import concourse.mybir as mybir
    import concourse.tile as tile
    from concourse import bacc

    fp32 = mybir.dt.float32
    bf16 = mybir.dt.bfloat16
    AF = mybir.ActivationFunctionType

    nc = bacc.Bacc("TRN2", target_bir_lowering=False, debug=False)

    x_d = nc.dram_tensor("x", [bl, T, C], fp32, kind="ExternalInput")
    wA_d = nc.dram_tensor("wA", [128, 704], bf16, kind="ExternalInput")
    wB_d = nc.dram_tensor("wB", [65, 704], bf16, kind="ExternalInput")
    wpA_d = nc.dram_tensor("wpA", [128, 192], bf16, kind="ExternalInput")
    wpB_d = nc.dram_tensor("wpB", [65, 192], bf16, kind="ExternalInput")
    tril_d = nc.dram_tensor("trilR", [128, 12, 128], bf16, kind="ExternalInput")
    idr_d = nc.dram_tensor("identR", [128, 128], bf16, kind="ExternalInput")
    out_d = nc.dram_tensor("out", [bl, T, C], fp32, kind="ExternalOutput")

    with tile.TileContext(nc) as tc, ExitStack() as ctx:
        consts = ctx.enter_context(tc.tile_pool(name="consts", bufs=1))
        sbq = ctx.enter_context(tc.tile_pool(name="sbq", bufs=2))
        sbp = ctx.enter_context(tc.tile_pool(name="sbp", bufs=3))
        ps = ctx.enter_context(
            tc.tile_pool(name="ps", bufs=1, space=bass.MemorySpace.PSUM)
        )

        wA = consts.tile([128, 704], bf16)
        nc.sync.dma_start(wA[:], wA_d[:])
        wB = consts.tile([65, 704], bf16)
        nc.sync.dma_start(wB[:], wB_d[:])
        wpA = consts.tile([128, 192], bf16)
        nc.sync.dma_start(wpA[:], wpA_d[:])
        wpB = consts.tile([65, 192], bf16)
        nc.sync.dma_start(wpB[:], wpB_d[:])
        trilR = consts.tile([128, 12, 128], bf16)
        nc.sync.dma_start(trilR[:], tril_d[:])
        ident = consts.tile([128, 128], bf16)
        nc.sync.dma_start(ident[:], idr_d[:])

        # S psum scatter: (ee, h) -> (bank, slot) with bank = h%4 = the PE
        # row-group of that S matmul. Concurrent row-tiled matmuls (different
        # row-groups) then always target distinct psum banks; same-bank
        # matmuls share a row-group and are serialized by the PE.
        # pidx: position in the packed [128, 12, 128] P16 layout.
        SMAP = {}
        fill = [0, 0, 0, 0]
        base = [0, 4, 8, 10]
        for ee in range(2):
            for h in range(NH):
                b = h % 4
                SMAP[(ee, h)] = (b, fill[b], base[b] + fill[b])
                fill[b] += 1

        def pt(tag, shape, dtype=fp32, name=None):
            return ps.tile(shape, dtype, tag=tag, name=name or f"ps_{tag}")

        for q in range(bl // 4):
            # ---------------- quad phase: load, transpose, qkv ----------
            xf = sbq.tile([128, 4, 192], fp32, tag="xf")
            nc.sync.dma_start(
                xf[:], x_d[4 * q : 4 * q + 4].rearrange("e t c -> t e c")
            )
            x16 = sbq.tile([128, 4, 256], bf16, tag="x16")
            nc.vector.tensor_copy(x16[:, :, 0:192], xf[:])

            xTp = pt("xt1", [128, 4, 2, 128], bf16)
            for e in range(4):
                nc.tensor.transpose(xTp[:, e, 0, :], x16[:, e, 0:128], ident[:])
                nc.tensor.transpose(xTp[:, e, 1, :], x16[:, e, 128:256], ident[:])
            xT = sbq.tile([128, 4, 2, 128], bf16, tag="xT")
            nc.vector.tensor_copy(xT[:], xTp[:])
            nc.gpsimd.memset(xT[64:65, :, 1, :], 1.0)

            # qkT j-blocks [q h0-3 | q h4-5 | k h0-3 | k h4-5], 4 banks
            T4 = pt("qs4", [128, 4, 4, 128])
            for j in range(4):
                nc.tensor.matmul(
                    T4[:, j, :, :],
                    wA[:, 128 * j : 128 * (j + 1)],
                    xT[:, :, 0, :],
                    start=True,
                    stop=False,
                )
                nc.tensor.matmul(
                    T4[:, j, :, :],
                    wB[:, 128 * j : 128 * (j + 1)],
                    xT[0:65, :, 1, :],
                    start=False,
                    stop=True,
                )
            qkT = sbq.tile([128, 4, 4, 128], bf16, tag="qkT")
            nc.scalar.copy(qkT[:], T4[:])

            v16 = sbq.tile([128, 4, 192], bf16, tag="v16")
            for g in range(2):
                vp = pt("vp2", [128, 2, 256], name=f"vp_{g}")
                for i in range(2):
                    e = 2 * g + i
                    nc.tensor.matmul(
                        vp[:, i, 0:192],
                        xT[:, e, 0, :],
                        wA[:, 512:704],
                        start=True,
                        stop=False,
                    )
                    nc.tensor.matmul(
                        vp[:, i, 0:192],
                        xT[0:65, e, 1, :],
                        wB[:, 512:704],
                        start=False,
                        stop=True,
                    )
                nc.scalar.copy(v16[:, 2 * g : 2 * g + 2, :], vp[:, :, 0:192])

            # ---------------- pair phase: attention core ----------------
            for half in range(2):
                e0 = 2 * half  # elems e0, e0+1 of this quad

                # S scattered per SMAP: [128, bank, slot, 128]
                S = pt("qs4", [128, 4, 4, 128], name=f"S_{half}")
                for ee in range(2):
                    e = e0 + ee
                    for h in range(NH):
                        r = (h % 4) * 32
                        jq, jk = (0, 2) if h < 4 else (1, 3)
                        b, sl, _ = SMAP[(ee, h)]
                        nc.tensor.matmul(
                            S[:, b, sl, :],
                            qkT[r : r + 32, jq, e, :],
                            qkT[r : r + 32, jk, e, :],
                            start=True,
                            stop=True,
                            tile_position=(r, 0),
                        )

                # P16/Pm/Pn packed [128, 12, 128] in pidx order
                P16 = sbp.tile([128, 12, 128], bf16, tag="P16")
                nc.scalar.activation(
                    P16[:, 0:8, :].rearrange("p (a b) s -> p a b s", a=2),
                    S[:, 0:2, :, :],
                    AF.Exp,
                )
                nc.scalar.activation(
                    P16[:, 8:12, :].rearrange("p (a b) s -> p a b s", a=2),
                    S[:, 2:4, 0:2, :],
                    AF.Exp,
                )

                Pm = sbp.tile([128, 12, 128], bf16, tag="Pm")
                nc.vector.tensor_mul(Pm[:], P16[:], trilR[:])
                rsum = sbp.tile([128, 12], fp32, tag="rsum")
                nc.vector.reduce_sum(rsum[:], Pm[:], axis=mybir.AxisListType.X)
                rrec2 = sbp.tile([128, 12, 2], fp32, tag="rrec2")
                nc.vector.reciprocal(
                    rrec2[:], rsum[:, :, None].broadcast_to([128, 12, 2])
                )
                Pn = sbp.tile([128, 12, 128], bf16, tag="Pn")
                nc.vector.tensor_mul(
                    Pn[:].rearrange("p a (c d) -> p a c d", d=2),
                    Pm[:].rearrange("p a (c d) -> p a c d", d=2),
                    rrec2[:, :, None, :].broadcast_to([128, 12, 64, 2]),
                )

                # transposes un-scatter: PTp/PT in canonical [h, ee] order
                PTp = pt("vp2", [128, 6, 2, 128], bf16, name=f"PTp_{half}")
                for ee in range(2):
                    for h in range(NH):
                        _, _, pidx = SMAP[(ee, h)]
                        nc.tensor.transpose(
                            PTp[:, h, ee, :], Pn[:, pidx, :], ident[:]
                        )
                PT = sbp.tile([128, 6, 2, 128], bf16, tag="PT")
                nc.scalar.copy(PT[:], PTp[:])

                yt = pt("yo1", [128, 2, 2, 128], name=f"yt_{half}")
                for ee in range(2):
                    e = e0 + ee
                    for h in range(NH):
                        r = (h % 4) * 32
                        j = 0 if h < 4 else 1
                        nc.tensor.matmul(
                            yt[r : r + 32, ee, j, :],
                            v16[:, e, h * 32 : h * 32 + 32],
                            PT[:, h, ee, :],
                            start=True,
                            stop=True,
                            tile_position=(0, r),
                        )
                yT = sbp.tile([128, 2, 2, 128], bf16, tag="yT")
                nc.vector.tensor_copy(yT[:, :, 0, :], yt[:, :, 0, :])
                nc.vector.tensor_copy(yT[0:64, :, 1, :], yt[0:64, :, 1, :])
                nc.gpsimd.memset(yT[64:65, :, 1, :], 1.0)

                outs = sbp.tile([128, 2, 192], fp32, tag="outs")
                outp = pt("yo1", [128, 2, 256], name=f"outp_{half}")
                for ee in range(2):
                    nc.tensor.matmul(
                        outp[:, ee, 0:192],
                        yT[:, ee, 0, :],
                        wpA[:],
                        start=True,
                        stop=False,
                    )
                    nc.tensor.matmul(
                        outp[:, ee, 0:192],
                        yT[0:65, ee, 1, :],
                        wpB[:],
                        start=False,
                        stop=True,
                    )
                nc.scalar.copy(outs[:], outp[:, :, 0:192])
                nc.sync.dma_start(
                    out_d[4 * q + e0 : 4 * q + e0 + 2].rearrange(
                        "e t c -> t e c"
                    ),
                    outs[:],
                )

    nc.finalize()
    return nc


def _prep_inputs(x, w_qkv, b_qkv, w_proj, b_proj, bl):
    bf = ml_dtypes.bfloat16
    scale = 1.0 / np.sqrt(HD)
    w2 = np.array(w_qkv, dtype=np.float32, copy=True)
    b2 = np.array(b_qkv, dtype=np.float32, copy=True)
    w2[:, 0:C] *= scale
    b2[0:C] *= scale
    # column order: [q h0-3 | q h4-5 + pad | k h0-3 | k h4-5 + pad | v]
    # (pad cols produce junk in unread partitions, keeping M=128 full-mode)
    perm = np.concatenate(
        [
            np.arange(0, 128),
            np.arange(128, 192),
            np.arange(0, 64),
            np.arange(192, 320),
            np.arange(320, 384),
            np.arange(0, 64),
            np.arange(384, 576),
        ]
    )
    wA = w2[0:128][:, perm].astype(bf)
    wB = np.concatenate([w2[128:192], b2[None, :]], axis=0)[:, perm].astype(bf)
    wpA = np.asarray(w_proj)[0:128].astype(bf)
    wpB = np.concatenate(
        [np.asarray(w_proj)[128:192], np.asarray(b_proj)[None, :]], axis=0
    ).astype(bf)
    trilR = np.ascontiguousarray(
        np.broadcast_to(
            np.tril(np.ones((128, 128), np.float32)), (12, 128, 128)
        ).transpose(1, 0, 2)
    ).astype(bf)
    identR = np.eye(128, dtype=np.float32).astype(bf)
    xs = np.ascontiguousarray(np.asarray(x, dtype=np.float32)).reshape(
        -1, bl, T, C
    )
    maps = []
    for i in range(xs.shape[0]):
        maps.append(
            {
                "x": xs[i],
                "wA": wA,
                "wB": wB,
                "wpA": wpA,
                "wpB": wpB,
                "trilR": trilR,
                "identR": identR,
            }
        )
    return maps


def _run(x, w_qkv, b_qkv, w_proj, b_proj, bl=BL, n_cores=N_CORES, trace=False):
    from concourse.bass_utils import run_bass_kernel_spmd

    key = bl
    if key not in _CACHE:
        _CACHE[key] = _build(bl)
    nc = _CACHE[key]
    maps = _prep_inputs(x, w_qkv, b_qkv, w_proj, b_proj, bl)[:n_cores]
    res = run_bass_kernel_spmd(
        nc, maps, core_ids=list(range(len(maps))), trace=trace
    )
    out = np.concatenate([r["out"] for r in res.results], axis=0)
    return out, res


def kernel(x, w_qkv, b_qkv, w_proj, b_proj):
    out, _ = _run(x, w_qkv, b_qkv, w_proj, b_proj)
    return out.reshape(B, T, C).astype(np.float32)
